# revision 3
# baseline (speedup 1.0000x reference)
"""MHA kernel for Trainium2, 8 NeuronCores.

Problem: B=4, S=2048, D=512, H=8 heads (head_dim 64).
  Q = x @ Wq.T ; K = x @ Wk.T ; V = x @ Wv.T  (per-head split)
  out = softmax(Q K^T / sqrt(512)) V          (concat heads)

Sharding: 8 cores = 4 batches x 2 head-groups (4 heads each).
Core c handles batch c//2, heads (c%2)*4 .. (c%2)*4+4.
Each core receives x[b] [2048,512] and the 256-row slices of Wq/Wk/Wv
for its heads, and produces y [2048,256] = out[b, :, g*256:(g+1)*256].
No collectives; the host scatters inputs and gathers outputs.

Per-core schedule (fp16 operands, fp32 PSUM/output):
  The wall floor is the ScalarE exp stream: 16.7M exps at 1 elem/
  cycle/partition (1.2GHz) + per-instruction access overhead ~= 134us
  busy. The PE stream (QK 131k + PV 131k + proj 49k + transposes ~16k
  cycles) just fits under it at ~2.4GHz, so everything is scheduled
  to keep ScalarE fed continuously from ~7us on:

  1. Warmup matmuls from t~=0 ramp the HAM clock governor inside the
     input-DMA window (the previous layout ran throttled 14-31us).
  2. DMA order: Wk bytes first, then x (16 tiles, triggers split
     sync/gpsimd), Wq/Wv triggered from ScalarE. Per 512-row x chunk:
     gpsimd casts f32->fp16, PE transposes, then K-pair0 AND Q-pair0
     projections for just that chunk — the first QK group issues once
     chunk 0 is in (~6us) and the exp stream starts while x DMA
     continues.
  3. Main loop per (pair, head, qc) block and kc-group: emit the 2-3
     QK matmuls, the exp for that group, then pop ~800ns of deferred
     PE work from a FIFO queue (remaining projections first, then PV
     kc-chunks which trail one exp-group behind, then O-transpose
     epilogues). Tensor's PSUM waits stay pre-satisfied (no exposed
     LDWEIGHTS stalls) and ScalarE never starves.
  4. PV accumulates V_aug^T E per block into one PSUM bank across the
     interleave (the ones-column yields softmax row sums for free);
     the epilogue PE-transposes O^T back, normalizes via DVE
     reciprocal, and DMAs the [128,4,64] output chunk per block.
"""

import os
import sys

import numpy as np

for _p in ("/opt/trn_rl_repo", "/root/.axon_site/_ro/trn_rl_repo"):
    if os.path.isdir(_p) and _p not in sys.path:
        sys.path.append(_p)

import concourse.bass as bass
import concourse.mybir as mybir
import concourse.tile as tile
from concourse import bacc
from concourse.bass_utils import run_bass_kernel_spmd
from concourse.masks import make_identity

F32 = mybir.dt.float32
FP16 = mybir.dt.float16

B, S, D, H = 4, 2048, 512, 8
HD = D // H          # 64
HL = 4               # heads per core
DQ = HL * HD         # 256 output dims per core
P = 128
DJ = D // P          # 4 contraction chunks
NT = S // P          # 16 s-tiles of 128
NQC = S // 512       # 4 q-chunks of 512
SCALE = 1.0 / float(np.sqrt(np.float32(D)))

# kc-groups for S^T psum/exp batching: (start, size) in 128-k-chunks
KC_GROUPS = [(0, 3), (3, 3), (6, 3), (9, 3), (12, 2), (14, 2)]

EXP = mybir.ActivationFunctionType.Exp


def build_nc():
    nc = bacc.Bacc("TRN2", target_bir_lowering=False, debug=False, num_devices=8)
    x = nc.dram_tensor("x", [S, D], F32, kind="ExternalInput")
    wq = nc.dram_tensor("wq", [DQ, D], F32, kind="ExternalInput")
    wk = nc.dram_tensor("wk", [DQ, D], F32, kind="ExternalInput")
    wv = nc.dram_tensor("wv", [DQ, D], F32, kind="ExternalInput")
    y = nc.dram_tensor("y", [S, DQ], F32, kind="ExternalOutput")

    with tile.TileContext(nc) as tc:
        with (
            tc.tile_pool(name="const", bufs=1) as cp,
            tc.tile_pool(name="xin", bufs=8) as xin,
            tc.tile_pool(name="win", bufs=6) as win,
            tc.tile_pool(name="ot", bufs=2) as otp,
            tc.tile_pool(name="ep", bufs=5) as ep,
            tc.tile_pool(name="pp", bufs=2, space="PSUM") as pp,
            tc.tile_pool(name="pq", bufs=2, space="PSUM") as pq,
        ):
            ident = cp.tile([P, P], F32)
            make_identity(nc, ident)
            identh = cp.tile([P, P], FP16)
            nc.vector.tensor_copy(identh[:], ident[:])

            xT = cp.tile([P, DJ, S], FP16)       # x.T  [d, s]
            wTs = {}
            for name in ("q", "k", "v"):
                wTs[name] = cp.tile([P, DJ, DQ], FP16, name=f"wT_{name}")
            QT = cp.tile([P, 2, S], FP16)        # head pair on partitions
            KT = cp.tile([P, 2, S], FP16)
            Vaug = cp.tile([P, NT, HL * (HD + 1)], FP16)  # V + ones cols
            Ofin = cp.tile([P, NT, DQ], F32)

            # PE warm-up during input DMA: ramps the HAM clock governor
            # so prologue compute is not throttled.
            wu = cp.tile([P, 512], FP16)
            nc.vector.memset(wu[:], 0.0)

            def warm(n):
                for _ in range(n):
                    pwu = pp.tile([P, 512], F32, tag="ps")
                    nc.tensor.matmul(
                        pwu[:], lhsT=wu[:, :P], rhs=wu[:], start=True, stop=True
                    )

            warm(4)
            nc.gpsimd.memset(Vaug[:], 1.0)

            # ---- DMA triggers: Wk bytes first, then x, Wq/Wv via ScalarE
            wtiles = {}
            for name, w in (("k", wk), ("q", wq), ("v", wv)):
                wtiles[name] = (
                    win.tile([P, D], F32, tag="w", name=f"w_{name}0"),
                    win.tile([P, D], F32, tag="w", name=f"w_{name}1"),
                )
            nc.sync.dma_start(wtiles["k"][0][:], wk[0:P, :])
            nc.sync.dma_start(wtiles["k"][1][:], wk[P : 2 * P, :])
            xtiles = []
            for i in range(16):
                t = xin.tile([P, D], F32, tag="x", name=f"x_{i}")
                dma_eng = nc.sync if i % 2 == 0 else nc.gpsimd
                dma_eng.dma_start(t[:], x[i * P : (i + 1) * P, :])
                xtiles.append(t)
            nc.scalar.dma_start(wtiles["q"][0][:], wq[0:P, :])
            nc.scalar.dma_start(wtiles["q"][1][:], wq[P : 2 * P, :])
            nc.scalar.dma_start(wtiles["v"][0][:], wv[0:P, :])
            nc.scalar.dma_start(wtiles["v"][1][:], wv[P : 2 * P, :])

            # ---- W casts + transposes (k first: K-proj is the critical path)
            for name in ("k", "q", "v"):
                wt0, wt1 = wtiles[name]
                wc0 = win.tile([P, D], FP16, tag="wc")
                wc1 = win.tile([P, D], FP16, tag="wc")
                nc.vector.tensor_copy(wc0[:], wt0[:])
                nc.scalar.copy(wc1[:], wt1[:])
                wcs = (wc0, wc1)
                G = pq.tile([P, 3, 512], F32, tag="G", name=f"Gw_{name}")
                Gh = G[:, 0, :].bitcast(FP16)  # [P, 1024] fp16 in one bank
                for j in range(DJ):
                    for p2 in range(2):
                        nc.tensor.transpose(
                            Gh[:, j * DQ + p2 * P : j * DQ + (p2 + 1) * P],
                            wcs[p2][:, j * P : (j + 1) * P],
                            identh,
                        )
                nc.vector.tensor_copy(
                    wTs[name][:, :, :],
                    Gh.rearrange("p (j c) -> p j c", j=DJ),
                )

            def proj_chain(dst_ap, wT, p2, sc, pool):
                if pool is pq:
                    gt = pq.tile([P, 3, 512], F32, tag="G")
                    pt = gt[:, 0, :]
                else:
                    t = pp.tile([P, 512], F32, tag="ps")
                    pt = t[:]
                for j in range(DJ):
                    nc.tensor.matmul(
                        pt,
                        lhsT=wT[:, j, p2 * P : (p2 + 1) * P],
                        rhs=xT[:, j, sc * 512 : (sc + 1) * 512],
                        start=(j == 0),
                        stop=(j == DJ - 1),
                    )
                nc.vector.tensor_copy(dst_ap, pt)

            # ---- x casts (gpsimd) + transposes + per-chunk K0/Q0 proj ----
            for tq in range(4):
                xcs = []
                for u in range(4):
                    xc = xin.tile([P, D], FP16, tag="xc")
                    nc.gpsimd.tensor_copy(xc[:], xtiles[tq * 4 + u][:])
                    xcs.append(xc)
                G = pq.tile([P, 3, 512], F32, tag="G", name=f"Gx_{tq}")
                for jj in range(2):
                    Gh = G[:, jj, :].bitcast(FP16)  # [P, 1024] fp16, one bank
                    for dj in range(2):
                        j = jj * 2 + dj
                        for u in range(4):
                            nc.tensor.transpose(
                                Gh[:, dj * 512 + u * P : dj * 512 + (u + 1) * P],
                                xcs[u][:, j * P : (j + 1) * P],
                                identh,
                            )
                    nc.vector.tensor_copy(
                        xT[:, jj * 2 : jj * 2 + 2, tq * 512 : (tq + 1) * 512],
                        Gh.rearrange("p (a b) -> p a b", a=2),
                    )
                # pair-0 K and Q projections for this s-chunk: only need the
                # xT columns transposed in this block, so the first QK group
                # can issue as soon as chunk 0 is in.
                proj_chain(KT[:, 0, tq * 512 : (tq + 1) * 512], wTs["k"], 0, tq, pp)
                proj_chain(QT[:, 0, tq * 512 : (tq + 1) * 512], wTs["q"], 0, tq, pp)
                if tq < 3:
                    warm(2)

            # ---- deferred-PE-work queue -----------------------------------
            # (cost_ns, fn) items; popped after each exp emission to fill the
            # tensor slack under the scalar exp pace.
            work_q = []

            def q_push(cost, fn):
                work_q.append((cost, fn))

            def v_chain(t):
                def emit():
                    gt = pq.tile([P, 3, 512], F32, tag="G")
                    pt = gt[:, 0, :]
                    for j in range(DJ):
                        nc.tensor.matmul(
                            pt[:, :DQ],
                            lhsT=xT[:, j, t * P : (t + 1) * P],
                            rhs=wTs["v"][:, j, :],
                            start=(j == 0),
                            stop=(j == DJ - 1),
                        )
                    vdst = Vaug[:, t, :].rearrange("p (h c) -> p h c", h=HL)[:, :, :HD]
                    vsrc = pt[:, :DQ].rearrange("p (h c) -> p h c", h=HL)
                    nc.vector.tensor_copy(vdst, vsrc)
                return emit

            # remaining projections in dependency-priority order: Q0 qc1-3
            # (blocks 1-3), V (first PV chunks), pair-1 K/Q (from block 8).
            for sc in range(1, NQC):
                q_push(900, (lambda sc=sc: proj_chain(
                    QT[:, 0, sc * 512 : (sc + 1) * 512], wTs["q"], 0, sc, pq)))
            for t in range(NT):
                q_push(470, v_chain(t))
            for sc in range(NQC):
                q_push(900, (lambda sc=sc: proj_chain(
                    KT[:, 1, sc * 512 : (sc + 1) * 512], wTs["k"], 1, sc, pq)))
            for sc in range(NQC):
                q_push(900, (lambda sc=sc: proj_chain(
                    QT[:, 1, sc * 512 : (sc + 1) * 512], wTs["q"], 1, sc, pq)))

            # ---- attention main loop --------------------------------------
            blocks = [(p2, e, qc) for p2 in (0, 1) for e in (0, 1) for qc in range(NQC)]
            yv = y[:].rearrange("(t p) c -> p t c", p=P)
            po_tiles = {}       # block idx -> accumulating PSUM tile (lazy)

            def pv_chunk(b, g0, gsz, E):
                p2, e, qc = blocks[b]
                hl = p2 * 2 + e

                def emit():
                    if b not in po_tiles:
                        po_tiles[b] = pp.tile([P, 512], F32, tag="ps", name=f"po_{b}")
                    po = po_tiles[b]
                    for i in range(gsz):
                        kc = g0 + i
                        nc.tensor.matmul(
                            po[: HD + 1, :],
                            lhsT=Vaug[:, kc, hl * (HD + 1) : (hl + 1) * (HD + 1)],
                            rhs=E[:, kc, :],
                            start=(kc == 0),
                            stop=(kc == NT - 1),
                        )
                return emit

            def epilogue(b):
                p2, e, qc = blocks[b]
                hl = p2 * 2 + e

                def emit():
                    po = po_tiles.pop(b)
                    ot = otp.tile([HD + 1, 512], F32, tag="ot")
                    nc.vector.tensor_copy(ot[:], po[: HD + 1, :])
                    pt = pp.tile([P, 512], F32, tag="ps", name=f"pt_{b}")
                    for u in range(4):
                        nc.tensor.transpose(
                            pt[:, u * (HD + 1) : (u + 1) * (HD + 1)],
                            ot[:, u * P : (u + 1) * P],
                            ident[: HD + 1, : HD + 1],
                        )
                    rt = otp.tile([P, 4], F32, tag="rt")
                    tv = pt[:, : 4 * (HD + 1)].rearrange("p (u c) -> p u c", u=4)
                    nc.vector.reciprocal(rt[:], tv[:, :, HD])
                    for u in range(4):
                        nc.vector.tensor_scalar_mul(
                            Ofin[:, qc * 4 + u, hl * HD : (hl + 1) * HD],
                            tv[:, u, :HD],
                            rt[:, u : u + 1],
                        )
                    nc.sync.dma_start(
                        yv[:, qc * 4 : (qc + 1) * 4, hl * HD : (hl + 1) * HD],
                        Ofin[:, qc * 4 : (qc + 1) * 4, hl * HD : (hl + 1) * HD],
                    )
                return emit

            def pop_work(budget):
                while work_q and budget > 0:
                    cost, fn = work_q.pop(0)
                    fn()
                    budget -= cost

            for b, (p2, e, qc) in enumerate(blocks):
                q0, q1 = qc * 512, (qc + 1) * 512
                E = ep.tile([P, NT, 512], FP16, tag="E", name=f"E_{b}")
                for g0, gsz in KC_GROUPS:
                    G = pq.tile([P, 3, 512], F32, tag="G", name=f"G_{b}_{g0}")
                    for i in range(gsz):
                        kc = g0 + i
                        nc.tensor.matmul(
                            G[:, i, :],
                            lhsT=KT[e * HD : (e + 1) * HD, p2, kc * P : (kc + 1) * P],
                            rhs=QT[e * HD : (e + 1) * HD, p2, q0:q1],
                            start=True,
                            stop=True,
                        )
                    nc.scalar.activation(
                        E[:, g0 : g0 + gsz, :], G[:, :gsz, :], EXP, scale=SCALE
                    )
                    # queue this group's PV chunk (unlocked by the exp above)
                    q_push(220 * gsz, pv_chunk(b, g0, gsz, E))
                    if g0 + gsz == NT:
                        q_push(300, epilogue(b))
                    pop_work(820 if gsz == 3 else 600)

            # drain: remaining PV chunks + epilogues
            pop_work(10**9)

    nc.compile()
    return nc


_NC_CACHE = None


def _get_nc():
    global _NC_CACHE
    if _NC_CACHE is None:
        _NC_CACHE = build_nc()
    return _NC_CACHE


def _in_maps(x, Wq, Wk, Wv):
    x = np.asarray(x, dtype=np.float32)
    Wq = np.asarray(Wq, dtype=np.float32)
    Wk = np.asarray(Wk, dtype=np.float32)
    Wv = np.asarray(Wv, dtype=np.float32)
    maps = []
    for c in range(8):
        b, g = c // 2, c % 2
        sl = slice(g * DQ, (g + 1) * DQ)
        maps.append(
            {
                "x": np.ascontiguousarray(x[b]),
                "wq": np.ascontiguousarray(Wq[sl]),
                "wk": np.ascontiguousarray(Wk[sl]),
                "wv": np.ascontiguousarray(Wv[sl]),
            }
        )
    return maps


def _install_trace_hook():
    """Register the NTFF profile hook that trn_agent_boot skipped
    (antenv.axon_hooks module is absent in this image). Test-only."""
    import types

    if "antenv.axon_hooks" in sys.modules:
        return
    from trn_agent_boot.trn_boot import _ntff_profile_via_ctypes

    hook = _ntff_profile_via_ctypes("/opt/axon/libaxon_pjrt.so")
    m = types.ModuleType("antenv.axon_hooks")
    m.get_axon_ntff_profile_hook = lambda: hook
    m.set_axon_ntff_profile_hook = lambda h: None
    sys.modules["antenv.axon_hooks"] = m
    import antenv

    antenv.axon_hooks = m


def run(x, Wq, Wk, Wv, trace=False):
    """Run on 8 cores; returns (full output [4,2048,512], BassKernelResults)."""
    if trace:
        _install_trace_hook()
    nc = _get_nc()
    res = run_bass_kernel_spmd(nc, _in_maps(x, Wq, Wk, Wv), list(range(8)), trace=trace)
    out = np.empty((B, S, D), dtype=np.float32)
    for c in range(8):
        b, g = c // 2, c % 2
        out[b, :, g * DQ : (g + 1) * DQ] = res.results[c]["y"]
    return out, res


def kernel(x, Wq, Wk, Wv):
    out, _ = run(x, Wq, Wk, Wv)
    return out


if __name__ == "__main__":
    rng = np.random.default_rng(0)
    x = rng.standard_normal((B, S, D)).astype(np.float32)
    sc = 1.0 / np.sqrt(D)
    Wq = rng.uniform(-sc, sc, (D, D)).astype(np.float32)
    Wk = rng.uniform(-sc, sc, (D, D)).astype(np.float32)
    Wv = rng.uniform(-sc, sc, (D, D)).astype(np.float32)
    out = kernel(x, Wq, Wk, Wv)
    print("ran", out.shape, out.dtype)


# revision 6
# speedup vs baseline: 1.0447x; 1.0447x over previous
"""MHA kernel for Trainium2, 8 NeuronCores.

Problem: B=4, S=2048, D=512, H=8 heads (head_dim 64).
  Q = x @ Wq.T ; K = x @ Wk.T ; V = x @ Wv.T  (per-head split)
  out = softmax(Q K^T / sqrt(512)) V          (concat heads)

Sharding: 8 cores = 4 batches x 2 head-groups (4 heads each).
Core c handles batch c//2, heads (c%2)*4 .. (c%2)*4+4.
Each core receives x[b] [2048,512] and the 256-row slices of Wq/Wk/Wv
for its heads, and produces y [2048,256] = out[b, :, g*256:(g+1)*256].
No collectives; the host scatters inputs and gathers outputs.

Per-core schedule (fp16 operands, fp32 PSUM/output):
  The wall floor is the ScalarE exp stream: 16.7M exps at 1 elem/
  cycle/partition (1.2GHz) + per-instruction access overhead ~= 134us
  busy. The PE stream (QK 131k + PV 131k + proj 49k + transposes ~16k
  cycles) just fits under it at full clock, so everything is
  scheduled to feed ScalarE continuously from ~10us on:

  1. Warmup matmuls from t~=0 keep the PE busy through the DMA window
     so the HAM clock governor starts ramping immediately (the PE
     runs at 1.2GHz until ~10-17us after sustained activity onset).
  2. DMA order: Wk bytes first, then x (16 tiles, triggers split
     sync/gpsimd), Wq/Wv triggered from ScalarE. Casts f32->fp16 are
     spread over Scalar (chunks 0-1, before its exp stream starts),
     DVE, and GpSimd (chunks 2-3; gpsimd casts are ~3x slower, so
     only where DVE is the prologue constraint).
  3. Per 512-row x chunk c: PE transposes, K-pair0 AND Q-pair0(qc=c)
     projections, then the QK score groups + exps of blocks 0..c
     whose kc-chunks are already resident (lexicographic (block,
     group) order — required for PSUM-ring safety). The exp stream
     starts once chunk 0 is in, while x DMA continues.
  4. Main loop per (pair, head, qc) block and kc-group: emit the 2-3
     QK matmuls, the exp, then pop deferred PE work from a FIFO queue
     under a debt-banked time budget (V/pair-1 projections first,
     then PV kc-chunks which trail one exp-group behind, then
     O-transpose epilogues). Tensor's PSUM waits stay pre-satisfied
     (no exposed LDWEIGHTS stalls) and ScalarE never starves.
  5. PV accumulates V_aug^T E per block into one PSUM bank across the
     interleave (the ones-column yields softmax row sums for free);
     the epilogue PE-transposes O^T back, normalizes via DVE
     reciprocal, and DMAs the [128,4,64] output chunk per block.
"""

import os
import sys

import numpy as np

for _p in ("/opt/trn_rl_repo", "/root/.axon_site/_ro/trn_rl_repo"):
    if os.path.isdir(_p) and _p not in sys.path:
        sys.path.append(_p)

import concourse.bass as bass
import concourse.mybir as mybir
import concourse.tile as tile
from concourse import bacc
from concourse.bass_utils import run_bass_kernel_spmd
from concourse.masks import make_identity

F32 = mybir.dt.float32
FP16 = mybir.dt.float16

B, S, D, H = 4, 2048, 512, 8
HD = D // H          # 64
HL = 4               # heads per core
DQ = HL * HD         # 256 output dims per core
P = 128
DJ = D // P          # 4 contraction chunks
NT = S // P          # 16 s-tiles of 128
NQC = S // 512       # 4 q-chunks of 512
SCALE = 1.0 / float(np.sqrt(np.float32(D)))

# kc-groups for S^T psum/exp batching: (start, size) in 128-k-chunks
KC_GROUPS = [(0, 3), (3, 3), (6, 3), (9, 3), (12, 2), (14, 2)]

EXP = mybir.ActivationFunctionType.Exp


def build_nc():
    nc = bacc.Bacc("TRN2", target_bir_lowering=False, debug=False, num_devices=8)
    x = nc.dram_tensor("x", [S, D], F32, kind="ExternalInput")
    wq = nc.dram_tensor("wq", [DQ, D], F32, kind="ExternalInput")
    wk = nc.dram_tensor("wk", [DQ, D], F32, kind="ExternalInput")
    wv = nc.dram_tensor("wv", [DQ, D], F32, kind="ExternalInput")
    y = nc.dram_tensor("y", [S, DQ], F32, kind="ExternalOutput")

    with tile.TileContext(nc) as tc:
        with (
            tc.tile_pool(name="const", bufs=1) as cp,
            tc.tile_pool(name="xin", bufs=8) as xin,
            tc.tile_pool(name="win", bufs=6) as win,
            tc.tile_pool(name="ot", bufs=2) as otp,
            tc.tile_pool(name="ep", bufs=5) as ep,
            tc.tile_pool(name="pp", bufs=2, space="PSUM") as pp,
            tc.tile_pool(name="pq", bufs=2, space="PSUM") as pq,
        ):
            ident = cp.tile([P, P], F32)
            make_identity(nc, ident)
            identh = cp.tile([P, P], FP16)
            nc.vector.tensor_copy(identh[:], ident[:])

            xT = cp.tile([P, DJ, S], FP16)       # x.T  [d, s]
            wTs = {}
            for name in ("q", "k", "v"):
                wTs[name] = cp.tile([P, DJ, DQ], FP16, name=f"wT_{name}")
            QT = cp.tile([P, 2, S], FP16)        # head pair on partitions
            KT = cp.tile([P, 2, S], FP16)
            Vaug = cp.tile([P, NT, HL * (HD + 1)], FP16)  # V + ones cols
            Ofin = cp.tile([P, NT, DQ], F32)

            # PE warm-up during input DMA: keeps activity high so the HAM
            # clock governor ramps as early as possible.
            wu = cp.tile([P, 512], FP16)
            nc.vector.memset(wu[:], 0.0)
            # only the ones-columns of Vaug need init; V bodies are written
            # by the projection evacs.
            ones_cols = Vaug[:].rearrange("p t (h c) -> p t h c", h=HL)[:, :, :, HD:]
            nc.vector.memset(ones_cols, 1.0)

            def warm(n):
                for _ in range(n):
                    pwu = pp.tile([P, 512], F32, tag="ps")
                    nc.tensor.matmul(
                        pwu[:], lhsT=wu[:, :P], rhs=wu[:], start=True, stop=True
                    )

            warm(6)

            # ---- DMA triggers: Wk bytes first, then x, Wq/Wv via ScalarE
            wtiles = {}
            for name in ("k", "q", "v"):
                wtiles[name] = (
                    win.tile([P, D], F32, tag="w", name=f"w_{name}0"),
                    win.tile([P, D], F32, tag="w", name=f"w_{name}1"),
                )
            nc.sync.dma_start(wtiles["k"][0][:], wk[0:P, :])
            nc.sync.dma_start(wtiles["k"][1][:], wk[P : 2 * P, :])
            xtiles = []
            for i in range(16):
                t = xin.tile([P, D], F32, tag="x", name=f"x_{i}")
                dma_eng = nc.sync if i % 2 == 0 else nc.gpsimd
                dma_eng.dma_start(t[:], x[i * P : (i + 1) * P, :])
                xtiles.append(t)
            nc.scalar.dma_start(wtiles["q"][0][:], wq[0:P, :])
            nc.scalar.dma_start(wtiles["q"][1][:], wq[P : 2 * P, :])
            nc.scalar.dma_start(wtiles["v"][0][:], wv[0:P, :])
            nc.scalar.dma_start(wtiles["v"][1][:], wv[P : 2 * P, :])

            # ---- W casts + transposes (k first: K-proj is the critical path)
            for name in ("k", "q", "v"):
                wt0, wt1 = wtiles[name]
                wc0 = win.tile([P, D], FP16, tag="wc")
                wc1 = win.tile([P, D], FP16, tag="wc")
                nc.vector.tensor_copy(wc0[:], wt0[:])
                nc.scalar.copy(wc1[:], wt1[:])
                wcs = (wc0, wc1)
                G = pq.tile([P, 3, 512], F32, tag="G", name=f"Gw_{name}")
                Gh = G[:, 0, :].bitcast(FP16)  # [P, 1024] fp16 in one bank
                for j in range(DJ):
                    for p2 in range(2):
                        nc.tensor.transpose(
                            Gh[:, j * DQ + p2 * P : j * DQ + (p2 + 1) * P],
                            wcs[p2][:, j * P : (j + 1) * P],
                            identh,
                        )
                nc.vector.tensor_copy(
                    wTs[name][:, :, :],
                    Gh.rearrange("p (j c) -> p j c", j=DJ),
                )

            def proj_chain(dst_ap, wT, p2, sc, pool):
                if pool is pq:
                    gt = pq.tile([P, 3, 512], F32, tag="G")
                    pt = gt[:, 0, :]
                else:
                    t = pp.tile([P, 512], F32, tag="ps")
                    pt = t[:]
                for j in range(DJ):
                    nc.tensor.matmul(
                        pt,
                        lhsT=wT[:, j, p2 * P : (p2 + 1) * P],
                        rhs=xT[:, j, sc * 512 : (sc + 1) * 512],
                        start=(j == 0),
                        stop=(j == DJ - 1),
                    )
                nc.vector.tensor_copy(dst_ap, pt)

            # ---- deferred-PE-work queue -----------------------------------
            work_q = []

            def q_push(cost, fn):
                work_q.append((cost, fn))

            blocks = [(p2, e, qc) for p2 in (0, 1) for e in (0, 1) for qc in range(NQC)]
            yv = y[:].rearrange("(t p) c -> p t c", p=P)
            po_tiles = {}       # block idx -> accumulating PSUM tile (lazy)
            E_tiles = {}        # block idx -> E SBUF tile (lazy)

            def pv_chunk(b, g0, gsz):
                p2, e, qc = blocks[b]
                hl = p2 * 2 + e
                E = E_tiles[b]

                def emit():
                    if b not in po_tiles:
                        po_tiles[b] = pp.tile([P, 512], F32, tag="ps", name=f"po_{b}")
                    po = po_tiles[b]
                    for i in range(gsz):
                        kc = g0 + i
                        nc.tensor.matmul(
                            po[: HD + 1, :],
                            lhsT=Vaug[:, kc, hl * (HD + 1) : (hl + 1) * (HD + 1)],
                            rhs=E[:, kc, :],
                            start=(kc == 0),
                            stop=(kc == NT - 1),
                        )
                return emit

            def epilogue(b):
                p2, e, qc = blocks[b]
                hl = p2 * 2 + e

                def emit():
                    po = po_tiles.pop(b)
                    ot = otp.tile([HD + 1, 512], F32, tag="ot")
                    nc.vector.tensor_copy(ot[:], po[: HD + 1, :])
                    pt = pp.tile([P, 512], F32, tag="ps", name=f"pt_{b}")
                    for u in range(4):
                        nc.tensor.transpose(
                            pt[:, u * (HD + 1) : (u + 1) * (HD + 1)],
                            ot[:, u * P : (u + 1) * P],
                            ident[: HD + 1, : HD + 1],
                        )
                    rt = otp.tile([P, 4], F32, tag="rt")
                    tv = pt[:, : 4 * (HD + 1)].rearrange("p (u c) -> p u c", u=4)
                    nc.vector.reciprocal(rt[:], tv[:, :, HD])
                    for u in range(4):
                        nc.vector.tensor_scalar_mul(
                            Ofin[:, qc * 4 + u, hl * HD : (hl + 1) * HD],
                            tv[:, u, :HD],
                            rt[:, u : u + 1],
                        )
                    nc.sync.dma_start(
                        yv[:, qc * 4 : (qc + 1) * 4, hl * HD : (hl + 1) * HD],
                        Ofin[:, qc * 4 : (qc + 1) * 4, hl * HD : (hl + 1) * HD],
                    )
                return emit

            def emit_qk(b, gi):
                """Emit the QK matmuls + exp for (block b, kc-group gi) and
                queue the matching PV chunk (+ epilogue after the last)."""
                p2, e, qc = blocks[b]
                g0, gsz = KC_GROUPS[gi]
                if b not in E_tiles:
                    E_tiles[b] = ep.tile([P, NT, 512], FP16, tag="E", name=f"E_{b}")
                E = E_tiles[b]
                q0, q1 = qc * 512, (qc + 1) * 512
                G = pq.tile([P, 3, 512], F32, tag="G", name=f"G_{b}_{g0}")
                for i in range(gsz):
                    kc = g0 + i
                    nc.tensor.matmul(
                        G[:, i, :],
                        lhsT=KT[e * HD : (e + 1) * HD, p2, kc * P : (kc + 1) * P],
                        rhs=QT[e * HD : (e + 1) * HD, p2, q0:q1],
                        start=True,
                        stop=True,
                    )
                nc.scalar.activation(
                    E[:, g0 : g0 + gsz, :], G[:, :gsz, :], EXP, scale=SCALE
                )
                q_push(225 * gsz, pv_chunk(b, g0, gsz))
                if g0 + gsz == NT:
                    q_push(310, epilogue(b))

            # V and pair-1 projections must be QUEUED before any PV chunk
            # (FIFO pop order = RAW order on Vaug/KT/QT); they are only
            # emitted at pop time, after the full x prologue.
            def v_chain(t):
                def emit():
                    gt = pq.tile([P, 3, 512], F32, tag="G")
                    pt = gt[:, 0, :]
                    for j in range(DJ):
                        nc.tensor.matmul(
                            pt[:, :DQ],
                            lhsT=xT[:, j, t * P : (t + 1) * P],
                            rhs=wTs["v"][:, j, :],
                            start=(j == 0),
                            stop=(j == DJ - 1),
                        )
                    vdst = Vaug[:, t, :].rearrange("p (h c) -> p h c", h=HL)[:, :, :HD]
                    vsrc = pt[:, :DQ].rearrange("p (h c) -> p h c", h=HL)
                    nc.vector.tensor_copy(vdst, vsrc)
                return emit

            for t in range(NT):
                q_push(470, v_chain(t))
            for sc in range(NQC):
                q_push(1060, (lambda sc=sc: proj_chain(
                    KT[:, 1, sc * 512 : (sc + 1) * 512], wTs["k"], 1, sc, pq)))
            for sc in range(NQC):
                q_push(1060, (lambda sc=sc: proj_chain(
                    QT[:, 1, sc * 512 : (sc + 1) * 512], wTs["q"], 1, sc, pq)))

            # ---- x casts + transposes + per-chunk K0/Q0 proj + early QK ---
            # group gi of block b<=3 is ready once x chunks 0..(3g+gsz-1)//4
            # are transposed; emit early groups lexicographically (PSUM-ring
            # safety requires strict (b, g) order).
            emitted = set()
            MAX_EARLY = 8

            def emit_ready(chunk_done):
                for b in range(chunk_done + 1):
                    for gi, (g0, gsz) in enumerate(KC_GROUPS):
                        if (b, gi) in emitted:
                            continue
                        if (g0 + gsz - 1) // 4 > chunk_done:
                            break
                        if len(emitted) >= MAX_EARLY:
                            return
                        emit_qk(b, gi)
                        emitted.add((b, gi))

            for tq in range(4):
                xcs = []
                for u in range(4):
                    xc = xin.tile([P, D], FP16, tag="xc")
                    if tq < 2:
                        eng = nc.scalar if u % 2 == 0 else nc.vector
                    else:
                        eng = nc.gpsimd if u % 2 == 0 else nc.vector
                    if eng is nc.scalar:
                        eng.copy(xc[:], xtiles[tq * 4 + u][:])
                    else:
                        eng.tensor_copy(xc[:], xtiles[tq * 4 + u][:])
                    xcs.append(xc)
                G = pq.tile([P, 3, 512], F32, tag="G", name=f"Gx_{tq}")
                for jj in range(2):
                    Gh = G[:, jj, :].bitcast(FP16)  # [P, 1024] fp16, one bank
                    for dj in range(2):
                        j = jj * 2 + dj
                        for u in range(4):
                            nc.tensor.transpose(
                                Gh[:, dj * 512 + u * P : dj * 512 + (u + 1) * P],
                                xcs[u][:, j * P : (j + 1) * P],
                                identh,
                            )
                    nc.vector.tensor_copy(
                        xT[:, jj * 2 : jj * 2 + 2, tq * 512 : (tq + 1) * 512],
                        Gh.rearrange("p (a b) -> p a b", a=2),
                    )
                # pair-0 K and Q(qc=tq) projections for this chunk; the QK
                # groups they unlock are emitted right behind them.
                proj_chain(KT[:, 0, tq * 512 : (tq + 1) * 512], wTs["k"], 0, tq, pp)
                proj_chain(QT[:, 0, tq * 512 : (tq + 1) * 512], wTs["q"], 0, tq, pp)
                if tq < 3:
                    emit_ready(tq)
                    warm(2)

            # ---- attention main loop --------------------------------------
            bank = [0.0]

            def pop_work(slot_budget):
                bank[0] = min(bank[0] + slot_budget, 1500.0)
                while work_q and bank[0] >= work_q[0][0]:
                    cost, fn = work_q.pop(0)
                    fn()
                    bank[0] -= cost

            for b in range(len(blocks)):
                for gi, (g0, gsz) in enumerate(KC_GROUPS):
                    if (b, gi) in emitted:
                        continue
                    emit_qk(b, gi)
                    pop_work(920 if gsz == 3 else 700)

            # drain: remaining PV chunks + epilogues
            while work_q:
                work_q.pop(0)[1]()

    nc.compile()
    return nc


_NC_CACHE = None


def _get_nc():
    global _NC_CACHE
    if _NC_CACHE is None:
        _NC_CACHE = build_nc()
    return _NC_CACHE


def _in_maps(x, Wq, Wk, Wv):
    x = np.asarray(x, dtype=np.float32)
    Wq = np.asarray(Wq, dtype=np.float32)
    Wk = np.asarray(Wk, dtype=np.float32)
    Wv = np.asarray(Wv, dtype=np.float32)
    maps = []
    for c in range(8):
        b, g = c // 2, c % 2
        sl = slice(g * DQ, (g + 1) * DQ)
        maps.append(
            {
                "x": np.ascontiguousarray(x[b]),
                "wq": np.ascontiguousarray(Wq[sl]),
                "wk": np.ascontiguousarray(Wk[sl]),
                "wv": np.ascontiguousarray(Wv[sl]),
            }
        )
    return maps


def _install_trace_hook():
    """Register the NTFF profile hook that trn_agent_boot skipped
    (antenv.axon_hooks module is absent in this image). Test-only."""
    import types

    if "antenv.axon_hooks" in sys.modules:
        return
    from trn_agent_boot.trn_boot import _ntff_profile_via_ctypes

    hook = _ntff_profile_via_ctypes("/opt/axon/libaxon_pjrt.so")
    m = types.ModuleType("antenv.axon_hooks")
    m.get_axon_ntff_profile_hook = lambda: hook
    m.set_axon_ntff_profile_hook = lambda h: None
    sys.modules["antenv.axon_hooks"] = m
    import antenv

    antenv.axon_hooks = m


def run(x, Wq, Wk, Wv, trace=False):
    """Run on 8 cores; returns (full output [4,2048,512], BassKernelResults)."""
    if trace:
        _install_trace_hook()
    nc = _get_nc()
    res = run_bass_kernel_spmd(nc, _in_maps(x, Wq, Wk, Wv), list(range(8)), trace=trace)
    out = np.empty((B, S, D), dtype=np.float32)
    for c in range(8):
        b, g = c // 2, c % 2
        out[b, :, g * DQ : (g + 1) * DQ] = res.results[c]["y"]
    return out, res


def kernel(x, Wq, Wk, Wv):
    out, _ = run(x, Wq, Wk, Wv)
    return out


if __name__ == "__main__":
    rng = np.random.default_rng(0)
    x = rng.standard_normal((B, S, D)).astype(np.float32)
    sc = 1.0 / np.sqrt(D)
    Wq = rng.uniform(-sc, sc, (D, D)).astype(np.float32)
    Wk = rng.uniform(-sc, sc, (D, D)).astype(np.float32)
    Wv = rng.uniform(-sc, sc, (D, D)).astype(np.float32)
    out = kernel(x, Wq, Wk, Wv)
    print("ran", out.shape, out.dtype)


# revision 9
# speedup vs baseline: 1.0495x; 1.0046x over previous
"""MHA kernel for Trainium2, 8 NeuronCores.

Problem: B=4, S=2048, D=512, H=8 heads (head_dim 64).
  Q = x @ Wq.T ; K = x @ Wk.T ; V = x @ Wv.T  (per-head split)
  out = softmax(Q K^T / sqrt(512)) V          (concat heads)

Sharding: 8 cores = 4 batches x 2 head-groups (4 heads each).
Core c handles batch c//2, heads (c%2)*4 .. (c%2)*4+4.
Each core receives x[b] [2048,512] and the 256-row slices of Wq/Wk/Wv
for its heads, and produces y [2048,256] = out[b, :, g*256:(g+1)*256].
No collectives; the host scatters inputs and gathers outputs.

Per-core schedule (fp16 operands, fp32 PSUM/output):
  The wall floor is the ScalarE exp stream: 16.7M exps at 1 elem/
  cycle/partition (1.2GHz) + per-instruction access overhead ~= 134us
  busy. The PE stream (QK 131k + PV 131k + proj 49k + transposes ~16k
  cycles) just fits under it at full clock, so everything is
  scheduled to feed ScalarE continuously from ~10us on:

  1. Warmup matmuls from t~=0 keep the PE busy through the DMA window
     so the HAM clock governor starts ramping immediately (the PE
     runs at 1.2GHz until ~10-17us after sustained activity onset).
  2. DMA order: Wk bytes first, then x (16 tiles, triggers split
     sync/gpsimd), Wq/Wv triggered from ScalarE. Casts f32->fp16 are
     spread over Scalar (chunks 0-1, before its exp stream starts),
     DVE, and GpSimd (chunks 2-3; gpsimd casts are ~3x slower, so
     only where DVE is the prologue constraint).
  3. Per 512-row x chunk c: PE transposes, K-pair0 AND Q-pair0(qc=c)
     projections, then the QK score groups + exps of blocks 0..c
     whose kc-chunks are already resident (lexicographic (block,
     group) order — required for PSUM-ring safety). The exp stream
     starts once chunk 0 is in, while x DMA continues.
  4. Main loop per (pair, head, qc) block and kc-group: emit the 2-3
     QK matmuls, the exp, then pop deferred PE work from a FIFO queue
     under a debt-banked time budget (V/pair-1 projections first,
     then PV kc-chunks which trail one exp-group behind, then
     O-transpose epilogues). Tensor's PSUM waits stay pre-satisfied
     (no exposed LDWEIGHTS stalls) and ScalarE never starves.
  5. PV accumulates V_aug^T E per block into one PSUM bank across the
     interleave (the ones-column yields softmax row sums for free);
     the epilogue PE-transposes O^T back, normalizes via DVE
     reciprocal, and DMAs the [128,4,64] output chunk per block.
"""

import os
import sys

import numpy as np

for _p in ("/opt/trn_rl_repo", "/root/.axon_site/_ro/trn_rl_repo"):
    if os.path.isdir(_p) and _p not in sys.path:
        sys.path.append(_p)

import concourse.bass as bass
import concourse.mybir as mybir
import concourse.tile as tile
from concourse import bacc
from concourse.bass_utils import run_bass_kernel_spmd
from concourse.masks import make_identity

F32 = mybir.dt.float32
FP16 = mybir.dt.float16

B, S, D, H = 4, 2048, 512, 8
HD = D // H          # 64
HL = 4               # heads per core
DQ = HL * HD         # 256 output dims per core
P = 128
DJ = D // P          # 4 contraction chunks
NT = S // P          # 16 s-tiles of 128
NQC = S // 512       # 4 q-chunks of 512
SCALE = 1.0 / float(np.sqrt(np.float32(D)))

# kc-groups for S^T psum/exp batching: (start, size) in 128-k-chunks
KC_GROUPS = [(0, 3), (3, 3), (6, 3), (9, 3), (12, 2), (14, 2)]

EXP = mybir.ActivationFunctionType.Exp


def build_nc():
    nc = bacc.Bacc("TRN2", target_bir_lowering=False, debug=False, num_devices=8)
    x = nc.dram_tensor("x", [S, D], F32, kind="ExternalInput")
    wq = nc.dram_tensor("wq", [DQ, D], F32, kind="ExternalInput")
    wk = nc.dram_tensor("wk", [DQ, D], F32, kind="ExternalInput")
    wv = nc.dram_tensor("wv", [DQ, D], F32, kind="ExternalInput")
    y = nc.dram_tensor("y", [S, DQ], F32, kind="ExternalOutput")

    with tile.TileContext(nc) as tc:
        with (
            tc.tile_pool(name="const", bufs=1) as cp,
            tc.tile_pool(name="xin", bufs=8) as xin,
            tc.tile_pool(name="win", bufs=6) as win,
            tc.tile_pool(name="ot", bufs=2) as otp,
            tc.tile_pool(name="ep", bufs=5) as ep,
            tc.tile_pool(name="pp", bufs=2, space="PSUM") as pp,
            tc.tile_pool(name="pq", bufs=2, space="PSUM") as pq,
        ):
            ident = cp.tile([P, P], F32)
            make_identity(nc, ident)
            identh = cp.tile([P, P], FP16)
            nc.vector.tensor_copy(identh[:], ident[:])

            xT = cp.tile([P, DJ, S], FP16)       # x.T  [d, s]
            wTs = {}
            for name in ("q", "k", "v"):
                wTs[name] = cp.tile([P, DJ, DQ], FP16, name=f"wT_{name}")
            QT = cp.tile([P, 2, S], FP16)        # head pair on partitions
            KT = cp.tile([P, 2, S], FP16)
            Vaug = cp.tile([P, NT, HL * (HD + 1)], FP16)  # V + ones cols
            Ofin = cp.tile([P, NT, DQ], F32)

            # PE warm-up during input DMA: keeps activity high so the HAM
            # clock governor ramps as early as possible.
            wu = cp.tile([P, 512], FP16)
            nc.vector.memset(wu[:], 0.0)
            # only the ones-columns of Vaug need init; V bodies are written
            # by the projection evacs.
            ones_cols = Vaug[:].rearrange("p t (h c) -> p t h c", h=HL)[:, :, :, HD:]
            nc.vector.memset(ones_cols, 1.0)

            def warm(n):
                for _ in range(n):
                    pwu = pp.tile([P, 512], F32, tag="ps")
                    nc.tensor.matmul(
                        pwu[:], lhsT=wu[:, :P], rhs=wu[:], start=True, stop=True
                    )

            warm(6)

            # ---- DMA triggers: Wk bytes first, then x, Wq/Wv via ScalarE
            wtiles = {}
            for name in ("k", "q", "v"):
                wtiles[name] = (
                    win.tile([P, D], F32, tag="w", name=f"w_{name}0"),
                    win.tile([P, D], F32, tag="w", name=f"w_{name}1"),
                )
            nc.sync.dma_start(wtiles["k"][0][:], wk[0:P, :])
            nc.sync.dma_start(wtiles["k"][1][:], wk[P : 2 * P, :])
            # Wq/Wv BEFORE x: they are small and must not queue behind the
            # 4MB x stream (scalar's cast chain would stall on them).
            nc.scalar.dma_start(wtiles["q"][0][:], wq[0:P, :])
            nc.scalar.dma_start(wtiles["q"][1][:], wq[P : 2 * P, :])
            nc.scalar.dma_start(wtiles["v"][0][:], wv[0:P, :])
            nc.scalar.dma_start(wtiles["v"][1][:], wv[P : 2 * P, :])
            xtiles = []
            for i in range(16):
                t = xin.tile([P, D], F32, tag="x", name=f"x_{i}")
                dma_eng = nc.sync if i % 2 == 0 else nc.gpsimd
                dma_eng.dma_start(t[:], x[i * P : (i + 1) * P, :])
                xtiles.append(t)

            # ---- W casts + transposes (k first: K-proj is the critical path)
            for name in ("k", "q", "v"):
                wt0, wt1 = wtiles[name]
                wc0 = win.tile([P, D], FP16, tag="wc")
                wc1 = win.tile([P, D], FP16, tag="wc")
                nc.vector.tensor_copy(wc0[:], wt0[:])
                nc.scalar.copy(wc1[:], wt1[:])
                wcs = (wc0, wc1)
                G = pq.tile([P, 3, 512], F32, tag="G", name=f"Gw_{name}")
                Gh = G[:, 0, :].bitcast(FP16)  # [P, 1024] fp16 in one bank
                for j in range(DJ):
                    for p2 in range(2):
                        nc.tensor.transpose(
                            Gh[:, j * DQ + p2 * P : j * DQ + (p2 + 1) * P],
                            wcs[p2][:, j * P : (j + 1) * P],
                            identh,
                        )
                nc.vector.tensor_copy(
                    wTs[name][:, :, :],
                    Gh.rearrange("p (j c) -> p j c", j=DJ),
                )

            def proj_chain(dst_ap, wT, p2, sc, pool):
                if pool is pq:
                    gt = pq.tile([P, 3, 512], F32, tag="G")
                    pt = gt[:, 0, :]
                else:
                    t = pp.tile([P, 512], F32, tag="ps")
                    pt = t[:]
                for j in range(DJ):
                    nc.tensor.matmul(
                        pt,
                        lhsT=wT[:, j, p2 * P : (p2 + 1) * P],
                        rhs=xT[:, j, sc * 512 : (sc + 1) * 512],
                        start=(j == 0),
                        stop=(j == DJ - 1),
                    )
                nc.vector.tensor_copy(dst_ap, pt)

            # ---- deferred-PE-work queue -----------------------------------
            work_q = []

            def q_push(cost, fn):
                work_q.append((cost, fn))

            blocks = [(p2, e, qc) for p2 in (0, 1) for e in (0, 1) for qc in range(NQC)]
            yv = y[:].rearrange("(t p) c -> p t c", p=P)
            po_tiles = {}       # block idx -> accumulating PSUM tile (lazy)
            E_tiles = {}        # block idx -> E SBUF tile (lazy)

            def pv_chunk(b, g0, gsz):
                p2, e, qc = blocks[b]
                hl = p2 * 2 + e
                E = E_tiles[b]

                def emit():
                    if b not in po_tiles:
                        po_tiles[b] = pp.tile([P, 512], F32, tag="ps", name=f"po_{b}")
                    po = po_tiles[b]
                    for i in range(gsz):
                        kc = g0 + i
                        nc.tensor.matmul(
                            po[: HD + 1, :],
                            lhsT=Vaug[:, kc, hl * (HD + 1) : (hl + 1) * (HD + 1)],
                            rhs=E[:, kc, :],
                            start=(kc == 0),
                            stop=(kc == NT - 1),
                        )
                return emit

            def epilogue(b):
                p2, e, qc = blocks[b]
                hl = p2 * 2 + e

                def emit():
                    po = po_tiles.pop(b)
                    ot = otp.tile([HD + 1, 512], F32, tag="ot")
                    nc.vector.tensor_copy(ot[:], po[: HD + 1, :])
                    pt = pp.tile([P, 512], F32, tag="ps", name=f"pt_{b}")
                    for u in range(4):
                        nc.tensor.transpose(
                            pt[:, u * (HD + 1) : (u + 1) * (HD + 1)],
                            ot[:, u * P : (u + 1) * P],
                            ident[: HD + 1, : HD + 1],
                        )
                    rt = otp.tile([P, 4], F32, tag="rt")
                    tv = pt[:, : 4 * (HD + 1)].rearrange("p (u c) -> p u c", u=4)
                    nc.vector.reciprocal(rt[:], tv[:, :, HD])
                    for u in range(4):
                        nc.vector.tensor_scalar_mul(
                            Ofin[:, qc * 4 + u, hl * HD : (hl + 1) * HD],
                            tv[:, u, :HD],
                            rt[:, u : u + 1],
                        )
                    nc.sync.dma_start(
                        yv[:, qc * 4 : (qc + 1) * 4, hl * HD : (hl + 1) * HD],
                        Ofin[:, qc * 4 : (qc + 1) * 4, hl * HD : (hl + 1) * HD],
                    )
                return emit

            def emit_qk(b, gi):
                """Emit the QK matmuls + exp for (block b, kc-group gi) and
                queue the matching PV chunk (+ epilogue after the last)."""
                p2, e, qc = blocks[b]
                g0, gsz = KC_GROUPS[gi]
                if b not in E_tiles:
                    E_tiles[b] = ep.tile([P, NT, 512], FP16, tag="E", name=f"E_{b}")
                E = E_tiles[b]
                q0, q1 = qc * 512, (qc + 1) * 512
                G = pq.tile([P, 3, 512], F32, tag="G", name=f"G_{b}_{g0}")
                for i in range(gsz):
                    kc = g0 + i
                    nc.tensor.matmul(
                        G[:, i, :],
                        lhsT=KT[e * HD : (e + 1) * HD, p2, kc * P : (kc + 1) * P],
                        rhs=QT[e * HD : (e + 1) * HD, p2, q0:q1],
                        start=True,
                        stop=True,
                    )
                nc.scalar.activation(
                    E[:, g0 : g0 + gsz, :], G[:, :gsz, :], EXP, scale=SCALE
                )
                q_push(225 * gsz, pv_chunk(b, g0, gsz))
                if g0 + gsz == NT:
                    q_push(400, epilogue(b))

            # V and pair-1 projections must be QUEUED before any PV chunk
            # (FIFO pop order = RAW order on Vaug/KT/QT); they are only
            # emitted at pop time, after the full x prologue.
            def v_chain(t):
                def emit():
                    gt = pq.tile([P, 3, 512], F32, tag="G")
                    pt = gt[:, 0, :]
                    for j in range(DJ):
                        nc.tensor.matmul(
                            pt[:, :DQ],
                            lhsT=xT[:, j, t * P : (t + 1) * P],
                            rhs=wTs["v"][:, j, :],
                            start=(j == 0),
                            stop=(j == DJ - 1),
                        )
                    vdst = Vaug[:, t, :].rearrange("p (h c) -> p h c", h=HL)[:, :, :HD]
                    vsrc = pt[:, :DQ].rearrange("p (h c) -> p h c", h=HL)
                    nc.vector.tensor_copy(vdst, vsrc)
                return emit

            for t in range(NT):
                q_push(700, v_chain(t))
            for sc in range(NQC):
                q_push(1300, (lambda sc=sc: proj_chain(
                    KT[:, 1, sc * 512 : (sc + 1) * 512], wTs["k"], 1, sc, pq)))
            for sc in range(NQC):
                q_push(1300, (lambda sc=sc: proj_chain(
                    QT[:, 1, sc * 512 : (sc + 1) * 512], wTs["q"], 1, sc, pq)))

            # ---- x casts + transposes + per-chunk K0/Q0 proj + early QK ---
            # group gi of block b<=3 is ready once x chunks 0..(3g+gsz-1)//4
            # are transposed; emit early groups lexicographically (PSUM-ring
            # safety requires strict (b, g) order).
            emitted = set()
            MAX_EARLY = 8

            def emit_ready(chunk_done):
                for b in range(chunk_done + 1):
                    for gi, (g0, gsz) in enumerate(KC_GROUPS):
                        if (b, gi) in emitted:
                            continue
                        if (g0 + gsz - 1) // 4 > chunk_done:
                            break
                        if len(emitted) >= MAX_EARLY:
                            return
                        emit_qk(b, gi)
                        emitted.add((b, gi))

            for tq in range(4):
                xcs = []
                for u in range(4):
                    xc = xin.tile([P, D], FP16, tag="xc")
                    if tq < 2:
                        eng = nc.scalar if u % 2 == 0 else nc.vector
                    else:
                        eng = nc.gpsimd if u % 2 == 0 else nc.vector
                    if eng is nc.scalar:
                        eng.copy(xc[:], xtiles[tq * 4 + u][:])
                    else:
                        eng.tensor_copy(xc[:], xtiles[tq * 4 + u][:])
                    xcs.append(xc)
                G = pq.tile([P, 3, 512], F32, tag="G", name=f"Gx_{tq}")
                for jj in range(2):
                    Gh = G[:, jj, :].bitcast(FP16)  # [P, 1024] fp16, one bank
                    for dj in range(2):
                        j = jj * 2 + dj
                        for u in range(4):
                            nc.tensor.transpose(
                                Gh[:, dj * 512 + u * P : dj * 512 + (u + 1) * P],
                                xcs[u][:, j * P : (j + 1) * P],
                                identh,
                            )
                    nc.vector.tensor_copy(
                        xT[:, jj * 2 : jj * 2 + 2, tq * 512 : (tq + 1) * 512],
                        Gh.rearrange("p (a b) -> p a b", a=2),
                    )
                # pair-0 K and Q(qc=tq) projections for this chunk; the QK
                # groups they unlock are emitted right behind them.
                proj_chain(KT[:, 0, tq * 512 : (tq + 1) * 512], wTs["k"], 0, tq, pp)
                proj_chain(QT[:, 0, tq * 512 : (tq + 1) * 512], wTs["q"], 0, tq, pp)
                if tq < 3:
                    emit_ready(tq)
                    warm(2)

            # ---- attention main loop --------------------------------------
            bank = [0.0]

            def pop_work(slot_budget):
                bank[0] = min(bank[0] + slot_budget, 1400.0)
                while work_q and bank[0] >= work_q[0][0]:
                    cost, fn = work_q.pop(0)
                    fn()
                    bank[0] -= cost

            for b in range(len(blocks)):
                for gi, (g0, gsz) in enumerate(KC_GROUPS):
                    if (b, gi) in emitted:
                        continue
                    emit_qk(b, gi)
                    pop_work(920 if gsz == 3 else 700)

            # drain: remaining PV chunks + epilogues
            while work_q:
                work_q.pop(0)[1]()

    nc.compile()
    return nc


_NC_CACHE = None


def _get_nc():
    global _NC_CACHE
    if _NC_CACHE is None:
        _NC_CACHE = build_nc()
    return _NC_CACHE


def _in_maps(x, Wq, Wk, Wv):
    x = np.asarray(x, dtype=np.float32)
    Wq = np.asarray(Wq, dtype=np.float32)
    Wk = np.asarray(Wk, dtype=np.float32)
    Wv = np.asarray(Wv, dtype=np.float32)
    maps = []
    for c in range(8):
        b, g = c // 2, c % 2
        sl = slice(g * DQ, (g + 1) * DQ)
        maps.append(
            {
                "x": np.ascontiguousarray(x[b]),
                "wq": np.ascontiguousarray(Wq[sl]),
                "wk": np.ascontiguousarray(Wk[sl]),
                "wv": np.ascontiguousarray(Wv[sl]),
            }
        )
    return maps


def _install_trace_hook():
    """Register the NTFF profile hook that trn_agent_boot skipped
    (antenv.axon_hooks module is absent in this image). Test-only."""
    import types

    if "antenv.axon_hooks" in sys.modules:
        return
    from trn_agent_boot.trn_boot import _ntff_profile_via_ctypes

    hook = _ntff_profile_via_ctypes("/opt/axon/libaxon_pjrt.so")
    m = types.ModuleType("antenv.axon_hooks")
    m.get_axon_ntff_profile_hook = lambda: hook
    m.set_axon_ntff_profile_hook = lambda h: None
    sys.modules["antenv.axon_hooks"] = m
    import antenv

    antenv.axon_hooks = m


def run(x, Wq, Wk, Wv, trace=False):
    """Run on 8 cores; returns (full output [4,2048,512], BassKernelResults)."""
    if trace:
        _install_trace_hook()
    nc = _get_nc()
    res = run_bass_kernel_spmd(nc, _in_maps(x, Wq, Wk, Wv), list(range(8)), trace=trace)
    out = np.empty((B, S, D), dtype=np.float32)
    for c in range(8):
        b, g = c // 2, c % 2
        out[b, :, g * DQ : (g + 1) * DQ] = res.results[c]["y"]
    return out, res


def kernel(x, Wq, Wk, Wv):
    out, _ = run(x, Wq, Wk, Wv)
    return out


if __name__ == "__main__":
    rng = np.random.default_rng(0)
    x = rng.standard_normal((B, S, D)).astype(np.float32)
    sc = 1.0 / np.sqrt(D)
    Wq = rng.uniform(-sc, sc, (D, D)).astype(np.float32)
    Wk = rng.uniform(-sc, sc, (D, D)).astype(np.float32)
    Wv = rng.uniform(-sc, sc, (D, D)).astype(np.float32)
    out = kernel(x, Wq, Wk, Wv)
    print("ran", out.shape, out.dtype)
